# revision 2
# baseline (speedup 1.0000x reference)
"""Hypernetwork causal attention (nn_Attention_87926570484382) on 8 TRN2 cores.

Strategy (single launch, batch-sharded attention, host-generated weights):
  host   : time-embedding MLP -> t [128]; W_attn/W_proj = t . fW; bias
           assembly; 1/sqrt(D) folded into q columns; x transposed per core.
  device : each core runs full attention for 2 of the 16 batches.
  v2 structural changes vs v1:
    - input DMAs partition-split into [32, W] slices across the 16 queues
      (DMA descriptors are per partition-row, ~70ns each: a [128, W] DMA
      costs ~9.3us on one queue regardless of W; [32, W] x4 costs 2.3us).
      First matmul dependency lands ~2.4us instead of ~12us.
    - vinit DMA (4MB of constant ones) replaced by on-chip DVE memsets.
    - scores psum = [128, 1024] two-bank tiles holding BOTH heads of a
      pair (h0 in bank0, h1 in bank1) -> ONE exp per (pair, j, qc) with a
      3D AP over both banks: 96 EXPs instead of 192 (saves ~19us ACT).
    - attnV psum = ONE [128, 2048] four-bank tile per pair: o rows 0-63,
      softmax denominators rows 64-127 (ones-extended v stationaries),
      heads side by side -> reciprocal-normalize becomes 5 wide DVE ops
      per pair ([64, 2048]) instead of 16 ops ([64, 512]) (saves ~35us).
    - qk/v/proj units merged to [128, 1024] psum slots: half the
      evacuation instructions, at double width.
    - output DMAs partition-split x4.
"""

import os
import sys

import numpy as np

# ---------------------------------------------------------------------------
# Environment shims (must precede concourse imports in fresh environments)
# ---------------------------------------------------------------------------


def _ensure_axon_hooks():
    """Provide antenv.axon_hooks if the installed antenv lacks it (needed
    only when tracing; harmless otherwise)."""
    try:
        import antenv.axon_hooks  # noqa: F401
        return
    except ImportError:
        pass
    try:
        import antenv
    except ImportError:
        return
    import contextlib
    import ctypes
    import types

    mod = types.ModuleType("antenv.axon_hooks")
    mod._HOOK = None
    mod._TRIED = False

    def set_axon_ntff_profile_hook(hook):
        mod._HOOK = hook

    def _build(so_path):
        lib = ctypes.CDLL(so_path)
        if not hasattr(lib, "axon_start_nrt_profile"):
            return None
        lib.axon_start_nrt_profile.argtypes = [
            ctypes.POINTER(ctypes.c_int64),
            ctypes.c_size_t,
        ]
        lib.axon_start_nrt_profile.restype = ctypes.c_int64
        lib.axon_stop_nrt_profile.argtypes = [ctypes.c_char_p]
        lib.axon_stop_nrt_profile.restype = ctypes.c_int64

        @contextlib.contextmanager
        def _hook(output_dir, device_ids):
            import jax

            jax.devices()

            def _start():
                if device_ids:
                    ids = (ctypes.c_int64 * len(device_ids))(*device_ids)
                    return lib.axon_start_nrt_profile(ids, len(device_ids))
                return lib.axon_start_nrt_profile(None, 0)

            rc = _start()
            if rc != 0:
                try:
                    lib.axon_stop_nrt_profile(str(output_dir).encode())
                except Exception:
                    pass
                rc = _start()
            started = rc == 0
            if not started:
                print(f"profile start failed rc={rc}; running untraced",
                      file=sys.stderr)
            try:
                yield
            finally:
                if started:
                    n = lib.axon_stop_nrt_profile(str(output_dir).encode())
                    print(f"profile: {n} file(s) -> {output_dir}",
                          file=sys.stderr)

        return _hook

    def get_axon_ntff_profile_hook():
        if mod._HOOK is None and not mod._TRIED:
            mod._TRIED = True
            p = "/opt/axon/libaxon_pjrt.so"
            if os.path.exists(p):
                try:
                    mod._HOOK = _build(p)
                except OSError:
                    mod._HOOK = None
        return mod._HOOK

    mod.set_axon_ntff_profile_hook = set_axon_ntff_profile_hook
    mod.get_axon_ntff_profile_hook = get_axon_ntff_profile_hook
    sys.modules["antenv.axon_hooks"] = mod
    antenv.axon_hooks = mod


_ensure_axon_hooks()

import concourse.bass as bass  # noqa: E402
import concourse.mybir as mybir  # noqa: E402
from concourse import tile as _tile  # noqa: E402
from concourse.tile import TileContext  # noqa: E402
from concourse.vector_clock import ScopedClock  # noqa: E402
from concourse.bass_utils import run_bass_kernel_spmd  # noqa: E402

F32 = mybir.dt.float32
F16 = mybir.dt.float16
I32 = mybir.dt.int32
RECIP_MAGIC = float(0x7EF311C3)  # fast-inverse seed constant

# problem constants (hardcoded per harness contract)
SIN_DIM, TEMBED = 64, 128
E, H, D = 512, 8, 64
B, S = 16, 1024
NCORES = 8
BPC = B // NCORES          # batches per core
S2 = BPC * S               # 2048 rows per core
J3 = 3 * H * D             # 1536
NET = E // 128             # 4 contraction tiles
NQT = S // 128             # 8 k/q tiles per batch

# ---------------------------------------------------------------------------
# Tile framework workarounds: this walrus accepts at most ONE semaphore wait
# and one update per instruction.
# ---------------------------------------------------------------------------

_NOP_CTR = [0]


def _patched_drain_and_barrier(self, tick_clock, wait_clock):
    carrier = self.nc.sync.nop(nofuse=True)
    wait_clock.add_sem_waits(
        carrier.ins, ScopedClock({None: tick_clock.global_clock})
    )
    si = carrier.ins.sync_info
    waits = list(si.on_wait) if si and si.on_wait else []
    if len(waits) > 1:
        carrier.ins.sync_info = mybir.SyncInfo(
            on_wait=waits[:1],
            on_update=list(si.on_update) if si and si.on_update else [],
        )
        for w in waits[1:]:
            extra = self.nc.sync.nop(nofuse=True)
            extra.ins.sync_info = mybir.SyncInfo(on_wait=[w], on_update=[])
    self.nc.sync.drain()
    self.nc.all_engine_barrier()
    assert self.sems is not None
    popped = self.nc._tile_sem_poison_stack.pop()
    assert popped is self._sem_poison
    self.nc.clear_and_free_semaphores(list(self.sems.allocated().values()))
    self.nc.all_engine_barrier()


_tile.TileContext._drain_and_barrier = _patched_drain_and_barrier


def _split_multi_waits(nc):
    for f in nc.m.functions:
        for blk in f.blocks:
            out = []
            changed = False
            for inst in blk.instructions:
                si = inst.sync_info
                waits = list(si.on_wait) if si and si.on_wait else []
                updates = list(si.on_update) if si and si.on_update else []
                is_dma = "DMA" in type(inst).__name__
                if len(waits) > 1:
                    changed = True
                    for w in waits[:-1]:
                        _NOP_CTR[0] += 1
                        nop = mybir.InstNoOp(
                            name=f"wsplit_{_NOP_CTR[0]}", ins=[], outs=[]
                        )
                        nop.engine = inst.engine
                        nop.sync_info = mybir.SyncInfo(on_wait=[w], on_update=[])
                        out.append(nop)
                    waits = [waits[-1]]
                    inst.sync_info = mybir.SyncInfo(
                        on_wait=waits, on_update=updates
                    )
                out.append(inst)
                if len(updates) > 1:
                    if is_dma:
                        raise AssertionError(
                            f"DMA {inst.name} has {len(updates)} updates"
                        )
                    changed = True
                    inst.sync_info = mybir.SyncInfo(
                        on_wait=waits, on_update=[updates[0]]
                    )
                    for u in updates[1:]:
                        _NOP_CTR[0] += 1
                        nop = mybir.InstNoOp(
                            name=f"usplit_{_NOP_CTR[0]}", ins=[], outs=[]
                        )
                        nop.engine = inst.engine
                        nop.sync_info = mybir.SyncInfo(on_wait=[], on_update=[u])
                        out.append(nop)
            if changed:
                blk.instructions = out
    return nc


# ---------------------------------------------------------------------------
# Single launch: attention for 2 batches per core
# ---------------------------------------------------------------------------


def build_attn(split_waits=True, salt=None):
    nc = bass.Bass()
    xt = nc.dram_tensor("xt", [E, S2], F16, kind="ExternalInput")
    wa = nc.dram_tensor("wa", [E, J3], F16, kind="ExternalInput")
    wp = nc.dram_tensor("wp", [E, E], F16, kind="ExternalInput")
    bqk = nc.dram_tensor("bqk", [128, 8], F32, kind="ExternalInput")
    bcol = nc.dram_tensor("bcol", [128, 4], F32, kind="ExternalInput")
    mask2 = nc.dram_tensor("mask2", [128, 256], F16, kind="ExternalInput")
    out = nc.dram_tensor("out", [E, S2], F16, kind="ExternalOutput")

    EXP = mybir.ActivationFunctionType.Exp
    IDENT = mybir.ActivationFunctionType.Identity

    with TileContext(nc) as tc:
        with (
            tc.tile_pool(name="cst", bufs=1) as cst,
            tc.tile_pool(name="qk", bufs=1) as qkp,
            tc.tile_pool(name="vx", bufs=1) as vxp,
            tc.tile_pool(name="ot", bufs=1) as otp,
            tc.tile_pool(name="ex", bufs=10) as exp_pool,
            tc.tile_pool(name="nr", bufs=3) as nrp,
            tc.tile_pool(name="ob", bufs=3) as obp,
            tc.tile_pool(name="ps", bufs=2, space="PSUM") as ps,
            tc.tile_pool(name="po", bufs=1, space="PSUM") as pop,
        ):
            # ---- SBUF tiles ----
            was = [
                cst.tile([128, J3], F16, tag=f"wa{et}", name=f"wa{et}")
                for et in range(NET)
            ]
            xts = [
                cst.tile([128, S2], F16, tag=f"xt{et}", name=f"xt{et}")
                for et in range(NET)
            ]
            wps = [
                cst.tile([128, E], F16, tag=f"wp{et}", name=f"wp{et}")
                for et in range(NET)
            ]
            bqk_t = cst.tile([128, 8], F32)
            bcol_t = cst.tile([128, 4], F32)
            mask2_t = cst.tile([128, 256], F16)
            # v_ext tiles: one per (batch, pair-of-k-tiles): layout
            # [128 k-rows, 2 st x 8 h x (64 v | 64 ones)]
            vxs = {
                (b, a): vxp.tile(
                    [128, 2048], F16, tag=f"vx{b}_{a}", name=f"vx{b}_{a}"
                )
                for b in range(BPC)
                for a in range(NQT // 2)
            }
            qkts = {
                (b, m): qkp.tile(
                    [128, S], F16, tag=f"qk{b}_{m}", name=f"qkT{b}_{m}"
                )
                for b in range(BPC)
                for m in range(8)
            }
            ots = {
                (b, i): otp.tile(
                    [128, S], F16, tag=f"ot{b}_{i}", name=f"oT{b}_{i}"
                )
                for b in range(BPC)
                for i in range(4)
            }

            # ---- input DMAs: [32, W] partition-slices, priority order ----
            def dma_in(tile_, src, c0, c1, pslices=4):
                step = 128 // pslices
                for a in range(pslices):
                    nc.sync.dma_start(
                        out=tile_[step * a : step * (a + 1), c0:c1],
                        in_=src[step * a : step * (a + 1), c0:c1],
                    )

            # R1: qk columns of wa + batch-0 halves of xt (first matmul
            # needs was[*][:, 0:128] and xts[*][:, 0:512])
            for et in range(NET):
                for a in range(4):
                    nc.sync.dma_start(
                        out=was[et][32 * a : 32 * (a + 1), 0:1024],
                        in_=wa[128 * et + 32 * a : 128 * et + 32 * (a + 1),
                               0:1024],
                    )
                    nc.sync.dma_start(
                        out=xts[et][32 * a : 32 * (a + 1), 0:1024],
                        in_=xt[128 * et + 32 * a : 128 * et + 32 * (a + 1),
                               0:1024],
                    )
            # R1.5: small tensors (biases + mask), partition-split
            for a in range(4):
                nc.sync.dma_start(
                    out=bqk_t[32 * a : 32 * (a + 1), :],
                    in_=bqk[32 * a : 32 * (a + 1), :],
                )
                nc.sync.dma_start(
                    out=mask2_t[32 * a : 32 * (a + 1), :],
                    in_=mask2[32 * a : 32 * (a + 1), :],
                )
            # R2: v columns of wa, batch-1 halves of xt, wp, bcol
            for et in range(NET):
                for a in range(4):
                    nc.sync.dma_start(
                        out=was[et][32 * a : 32 * (a + 1), 1024:1536],
                        in_=wa[128 * et + 32 * a : 128 * et + 32 * (a + 1),
                               1024:1536],
                    )
                    nc.sync.dma_start(
                        out=xts[et][32 * a : 32 * (a + 1), 1024:2048],
                        in_=xt[128 * et + 32 * a : 128 * et + 32 * (a + 1),
                               1024:2048],
                    )
            for et in range(NET):
                for a in range(4):
                    nc.sync.dma_start(
                        out=wps[et][32 * a : 32 * (a + 1), :],
                        in_=wp[128 * et + 32 * a : 128 * et + 32 * (a + 1), :],
                    )
            for a in range(4):
                nc.sync.dma_start(
                    out=bcol_t[32 * a : 32 * (a + 1), :],
                    in_=bcol[32 * a : 32 * (a + 1), :],
                )

            # ---- ones-columns of v_ext via on-chip memset (no DMA) ----
            for b in range(BPC):
                for a in range(NQT // 2):
                    ones_ap = (
                        vxs[(b, a)][:, :]
                        .rearrange("p (s h c) -> p s h c", s=2, h=H)[
                            :, :, :, 64:128
                        ]
                    )
                    nc.vector.memset(ones_ap, 1.0)

            # ---- work units ----
            emitted = set()

            def qk_unit(b, m):
                """qkT[m] (full 1024 q columns) for batch b (+ bias)."""
                pq0 = ps.tile([128, 1024], F32, tag="ps", name=f"pq_{b}_{m}")
                for et in range(NET):
                    for sc in range(2):
                        s0 = b * S + 512 * sc
                        nc.tensor.matmul(
                            pq0[:, 512 * sc : 512 * (sc + 1)],
                            was[et][:, 128 * m : 128 * (m + 1)],
                            xts[et][:, s0 : s0 + 512],
                            start=(et == 0), stop=(et == NET - 1),
                        )
                if m % 2 == 0:
                    nc.scalar.activation(
                        qkts[(b, m)][:, :], pq0,
                        func=IDENT, bias=bqk_t[:, m : m + 1],
                    )
                else:
                    nc.vector.tensor_scalar_add(
                        qkts[(b, m)][:, :], pq0, bqk_t[:, m : m + 1]
                    )
                emitted.add(("qk", b, m))

            def v_unit(b, a):
                """v rows for seq-tiles (2a, 2a+1) of batch b -> v_ext."""
                pv_ = ps.tile([128, 1024], F32, tag="ps", name=f"pv_{b}_{a}")
                for st in (2 * a, 2 * a + 1):
                    s0 = b * S + 128 * st
                    for et in range(NET):
                        nc.tensor.matmul(
                            pv_[:, 512 * (st % 2) : 512 * (st % 2 + 1)],
                            xts[et][:, s0 : s0 + 128],
                            was[et][:, 1024:1536],
                            start=(et == 0),
                            stop=(et == NET - 1),
                        )
                dst = (
                    vxs[(b, a)][:, :]
                    .rearrange("p (s h c) -> p s h c", s=2, h=H)[:, :, :, 0:64]
                )
                src = pv_[:, :].rearrange("p (s h c) -> p s h c", s=2, h=H)
                nc.vector.tensor_copy(dst, src)
                emitted.add(("v", b, a))

            def proj_unit(b, eb):
                """output e-rows [128*eb:...] for batch b (full 1024 cols)."""
                pp_ = ps.tile([128, 1024], F32, tag="ps", name=f"pp_{b}_{eb}")
                for i in range(4):
                    for sc in range(2):
                        nc.tensor.matmul(
                            pp_[:, 512 * sc : 512 * (sc + 1)],
                            wps[i][:, 128 * eb : 128 * (eb + 1)],
                            ots[(b, i)][:, 512 * sc : 512 * (sc + 1)],
                            start=(i == 0), stop=(i == 3),
                        )
                ob_ = obp.tile([128, 1024], F16, tag="ob", name=f"ob_{b}_{eb}")
                if eb % 2 == 0:
                    nc.scalar.activation(
                        ob_, pp_, func=IDENT, bias=bcol_t[:, eb : eb + 1]
                    )
                else:
                    nc.vector.tensor_scalar_add(
                        ob_, pp_, bcol_t[:, eb : eb + 1]
                    )
                s0 = b * S
                for a in range(4):
                    nc.sync.dma_start(
                        out=out[128 * eb + 32 * a : 128 * eb + 32 * (a + 1),
                                s0 : s0 + S],
                        in_=ob_[32 * a : 32 * (a + 1), :],
                    )

            def attn_pair(b, i, bg, slots=None):
                """Causal attention for heads (2i, 2i+1) of batch b."""
                for mm in (i, 4 + i):
                    assert ("qk", b, mm) in emitted
                kt = qkts[(b, 4 + i)]
                qt = qkts[(b, i)]
                exs = {
                    j: exp_pool.tile(
                        [128, 2 * S], F16, tag="ex", name=f"ex_{b}_{i}_{j}"
                    )
                    for j in range(NQT)
                }
                # po: [128, 2048] = heads side by side; per head: o rows
                # 0-63, denominators rows 64-127 (ones-extension); q cols
                # 0-1023 across two banks
                po = pop.tile([128, 2048], F32, tag="po", name=f"po_{b}_{i}")

                bg = list(bg)
                nbg = len(bg)
                if slots is None:
                    slots = [nbg // NQT + (1 if j < nbg % NQT else 0)
                             for j in range(NQT)]
                slots = list(slots)
                assert sum(slots) == nbg
                bi = 0

                def score_block(j, qlo, qhi):
                    """One [128,1024] psum slot: h0 scores in bank0,
                    h1 in bank1, for q columns [qlo:qhi); exp to exs[j]."""
                    w = qhi - qlo
                    s_ = ps.tile(
                        [128, 1024], F32, tag="ps", name=f"s_{b}_{i}_{j}_{qlo}"
                    )
                    nc.tensor.matmul(
                        s_[:, 0:w],
                        kt[0:64, 128 * j : 128 * (j + 1)],
                        qt[0:64, qlo:qhi],
                        start=True, stop=True,
                    )
                    nc.tensor.matmul(
                        s_[:, 512 : 512 + w],
                        kt[64:128, 128 * j : 128 * (j + 1)],
                        qt[64:128, qlo:qhi],
                        start=True, stop=True,
                    )
                    src = s_[:, :].rearrange("p (h c) -> p h c", h=2)[
                        :, :, 0:w
                    ]
                    dst = exs[j][:, :].rearrange("p (h c) -> p h c", h=2)[
                        :, :, qlo:qhi
                    ]
                    nc.scalar.activation(dst, src, func=EXP)

                def attnv_step(hh, j):
                    h = 2 * i + hh
                    stat = vxs[(b, j // 2)][
                        :, 1024 * (j % 2) + 128 * h : 1024 * (j % 2)
                        + 128 * (h + 1)
                    ]
                    if j < 4:
                        nc.tensor.matmul(
                            po[:, 1024 * hh + 128 * j : 1024 * hh + 512],
                            stat,
                            exs[j][:, 1024 * hh + 128 * j : 1024 * hh + 512],
                            start=(j == 0), stop=(j == 3),
                        )
                    c1 = max(512, 128 * j)
                    nc.tensor.matmul(
                        po[:, 1024 * hh + c1 : 1024 * hh + 1024],
                        stat,
                        exs[j][:, 1024 * hh + c1 : 1024 * hh + 1024],
                        start=(j == 0), stop=(j == NQT - 1),
                    )

                for j in range(NQT):
                    if j < 4:
                        score_block(j, 128 * j, 512)
                        # keep the PE fed between the two psum slots
                        if bi < nbg and slots[j] > 1:
                            bg[bi]()
                            bi += 1
                            slots[j] -= 1
                        score_block(j, 512, 1024)
                    else:
                        score_block(j, 128 * j, 1024)
                    # causal mask on the diagonal 128-col blocks (GpSimd:
                    # SBUF-only f16 work; 2D APs only)
                    nc.gpsimd.tensor_mul(
                        exs[j][:, 128 * j : 128 * (j + 1)],
                        exs[j][:, 128 * j : 128 * (j + 1)],
                        mask2_t[:, 0:128],
                    )
                    nc.gpsimd.tensor_mul(
                        exs[j][:, 1024 + 128 * j : 1024 + 128 * (j + 1)],
                        exs[j][:, 1024 + 128 * j : 1024 + 128 * (j + 1)],
                        mask2_t[:, 0:128],
                    )
                    attnv_step(0, j)
                    attnv_step(1, j)
                    for _ in range(slots[j]):
                        bg[bi]()
                        bi += 1
                assert bi == nbg

                # ---- normalize: 1/denominator via bit-trick seed + one
                # Newton step; denominators sit in po rows 64-127 ----
                sfx = f"{b}_{i}"
                s_ = nrp.tile([64, 2048], F32, tag="nr", name=f"nrs_{sfx}")
                t_ = nrp.tile([64, 2048], F32, tag="nr", name=f"nrt_{sfx}")
                u_ = nrp.tile([64, 2048], F32, tag="nr", name=f"nru_{sfx}")
                nc.vector.tensor_scalar(
                    out=s_[:, :].bitcast(I32),
                    in0=po[64:128, :].bitcast(I32),
                    scalar1=RECIP_MAGIC,
                    scalar2=-1.0,
                    op0=mybir.AluOpType.subtract,
                    op1=mybir.AluOpType.mult,
                )
                nc.vector.tensor_mul(t_, po[64:128, :], s_)
                nc.vector.scalar_tensor_tensor(
                    out=u_, in0=t_, scalar=2.0, in1=s_,
                    op0=mybir.AluOpType.subtract,
                    op1=mybir.AluOpType.mult,
                )
                for hh in range(2):
                    nc.vector.scalar_tensor_tensor(
                        out=ots[(b, i)][64 * hh : 64 * hh + 64, :],
                        in0=po[0:64, 1024 * hh : 1024 * (hh + 1)],
                        scalar=-1.0,
                        in1=u_[:, 1024 * hh : 1024 * (hh + 1)],
                        op0=mybir.AluOpType.mult,
                        op1=mybir.AluOpType.mult,
                    )

            # ---- schedule ----
            # lead-in: qk tiles for pair (0,0)
            qk_unit(0, 0)
            qk_unit(0, 4)

            def mkqk(b, m):
                return lambda: qk_unit(b, m)

            def mkv(b, a):
                return lambda: v_unit(b, a)

            def mkp(b, eb):
                return lambda: proj_unit(b, eb)

            # v(b,a) must be emitted before attnv_step of j=2a; the j<4
            # "between score blocks" slot runs before that j's attnV.
            bg_for_pair = {
                (0, 0): [mkv(0, 0), mkv(0, 1), mkqk(0, 1), mkv(0, 2),
                         mkqk(0, 5), mkv(0, 3)],
                (0, 1): [mkqk(0, 2), mkqk(0, 6)],
                (0, 2): [mkqk(0, 3), mkqk(0, 7)],
                (0, 3): [mkqk(1, 0), mkqk(1, 4)],
                (1, 0): [mkv(1, 0), mkv(1, 1), mkqk(1, 1), mkv(1, 2),
                         mkqk(1, 5), mkv(1, 3)],
                (1, 1): [mkqk(1, 2), mkqk(1, 6)],
                (1, 2): [mkqk(1, 3), mkqk(1, 7), mkp(0, 0), mkp(0, 1)],
                (1, 3): [mkp(0, 2), mkp(0, 3)],
            }
            slots_override = {
                (0, 0): [2, 1, 1, 2, 0, 0, 0, 0],
                (1, 0): [2, 1, 1, 2, 0, 0, 0, 0],
                (1, 2): [0, 1, 1, 1, 1, 0, 0, 0],
                (1, 3): [0, 0, 1, 1, 0, 0, 0, 0],
            }
            for b in range(BPC):
                for i in range(4):
                    attn_pair(
                        b, i, bg_for_pair[(b, i)],
                        slots=slots_override.get((b, i)),
                    )
            # tail: batch-1 projection
            for eb in range(4):
                proj_unit(1, eb)
    if salt:
        nop = mybir.InstNoOp(name=f"salt_{salt}", ins=[], outs=[])
        nop.engine = mybir.EngineType.SP
        nop.sync_info = mybir.SyncInfo(on_wait=[], on_update=[])
        nc.m.functions[0].blocks[0].instructions.append(nop)
    if split_waits:
        _split_multi_waits(nc)
    return nc


# ---------------------------------------------------------------------------
# Host orchestration
# ---------------------------------------------------------------------------

_CACHE = {}


def _get(name, builder):
    if name not in _CACHE:
        _CACHE[name] = builder()
    return _CACHE[name]


def _run_with_retry(nc, in_maps, trace=False, tries=3):
    import time as _time

    last = None
    for attempt in range(tries):
        try:
            return run_bass_kernel_spmd(
                nc, in_maps, core_ids=list(range(NCORES)), trace=trace
            )
        except Exception as e:  # transient NRT_EXEC_UNIT_UNRECOVERABLE etc.
            last = e
            _time.sleep(2.0 * (attempt + 1))
    raise last


def _silu(v):
    return v / (1.0 + np.exp(-v))


def kernel(
    time_embed,
    x,
    lin1_w,
    lin1_b,
    lin2_w,
    lin2_b,
    fW_attn_w,
    fW_attn_b,
    fb_attn,
    fW_proj_w,
    fW_proj_b,
    fb_proj,
    _trace=False,
    _times=None,
):
    f64 = np.float64
    # ---- host: time-embedding MLP ----
    t1 = _silu(time_embed.astype(f64) @ lin1_w.astype(f64) + lin1_b.astype(f64))
    t = t1 @ lin2_w.astype(f64) + lin2_b.astype(f64)   # [128]
    t32 = t.astype(np.float32)

    # ---- host: hypernetwork weights ----
    Wa = t32 @ fW_attn_w.reshape(TEMBED, E * J3)
    Wa = Wa.reshape(E, J3) + fW_attn_b.reshape(E, J3)
    Wp = t32 @ fW_proj_w.reshape(TEMBED, E * E)
    Wp = Wp.reshape(E, E) + fW_proj_b.reshape(E, E)
    Wa[:, :512] *= 0.125  # fold 1/sqrt(D) into q columns

    # ---- host: biases ----
    b_attn = (t @ fb_attn.astype(f64).reshape(TEMBED, J3)).astype(np.float32)
    bqk_host = b_attn[:1024].copy()
    bqk_host[:512] *= 0.125
    bqk_in = np.ascontiguousarray(bqk_host.reshape(8, 128).T)
    b_v = b_attn[1024:]
    b_proj = (t @ fb_proj.astype(f64)).astype(np.float32)
    brow = (b_v.astype(f64) @ Wp.astype(f64) + b_proj).astype(np.float32)
    bcol_in = np.ascontiguousarray(brow.reshape(4, 128).T)
    mask1 = np.triu(np.ones((128, 128), dtype=np.float16))
    mask2_in = np.ascontiguousarray(np.concatenate([mask1, mask1], axis=1))
    Wa16 = Wa.astype(np.float16)
    Wp16 = Wp.astype(np.float16)

    # ---- launch: attention ----
    nc_attn = _get("attn", build_attn)
    in_maps = []
    for c in range(NCORES):
        xt_c = np.ascontiguousarray(
            x[BPC * c : BPC * (c + 1)].reshape(S2, E).T
        )
        in_maps.append(
            {
                "xt": xt_c.astype(np.float16),
                "wa": Wa16,
                "wp": Wp16,
                "bqk": bqk_in,
                "bcol": bcol_in,
                "mask2": mask2_in,
            }
        )
    res = _run_with_retry(nc_attn, in_maps, trace=_trace)
    if _times is not None:
        _times.append(res.exec_time_ns)

    out = np.empty((B, S, E), dtype=np.float32)
    for c in range(NCORES):
        out[BPC * c : BPC * (c + 1)] = (
            res.results[c]["out"].astype(np.float32).T.reshape(BPC, S, E)
        )
    return out


# revision 3
# speedup vs baseline: 1.0638x; 1.0638x over previous
"""Hypernetwork causal attention (nn_Attention_87926570484382) on 8 TRN2 cores.

Strategy (single launch, batch-sharded attention, host-generated weights):
  host   : time-embedding MLP -> t [128]; W_attn/W_proj = t . fW; bias
           assembly; 1/sqrt(D) folded into q columns; x transposed per core.
           QK-generation operands packed to fp8e4 (x unscaled, Wa_qk x32,
           score descaling folded into the exp scale argument).
  device : each core runs full attention for 2 of the 16 batches.
  v3 changes vs v1 baseline:
    - QK generation in fp8 DoubleRow (K=256 packed into the array):
      64 matmuls instead of 128 -> ~18us less PE time.
    - attnV accumulates into ONE [128, 2048] psum tile per pair (heads
      side by side, o rows 0-63, softmax denominators rows 64-127) ->
      reciprocal-normalize is 5 wide DVE ops per pair.
    - vinit DMA (4MB of ones) replaced by on-chip DVE memsets.
    - input DMAs partition-split so no single queue serializes 128
      descriptor rows; critical first tensors land ~5us earlier.
    - output DMAs partition-split x4.
"""

import os
import sys

import numpy as np

# ---------------------------------------------------------------------------
# Environment shims (must precede concourse imports in fresh environments)
# ---------------------------------------------------------------------------


def _ensure_axon_hooks():
    """Provide antenv.axon_hooks if the installed antenv lacks it (needed
    only when tracing; harmless otherwise)."""
    try:
        import antenv.axon_hooks  # noqa: F401
        return
    except ImportError:
        pass
    try:
        import antenv
    except ImportError:
        return
    import contextlib
    import ctypes
    import types

    mod = types.ModuleType("antenv.axon_hooks")
    mod._HOOK = None
    mod._TRIED = False

    def set_axon_ntff_profile_hook(hook):
        mod._HOOK = hook

    def _build(so_path):
        lib = ctypes.CDLL(so_path)
        if not hasattr(lib, "axon_start_nrt_profile"):
            return None
        lib.axon_start_nrt_profile.argtypes = [
            ctypes.POINTER(ctypes.c_int64),
            ctypes.c_size_t,
        ]
        lib.axon_start_nrt_profile.restype = ctypes.c_int64
        lib.axon_stop_nrt_profile.argtypes = [ctypes.c_char_p]
        lib.axon_stop_nrt_profile.restype = ctypes.c_int64

        @contextlib.contextmanager
        def _hook(output_dir, device_ids):
            import jax

            jax.devices()

            def _start():
                if device_ids:
                    ids = (ctypes.c_int64 * len(device_ids))(*device_ids)
                    return lib.axon_start_nrt_profile(ids, len(device_ids))
                return lib.axon_start_nrt_profile(None, 0)

            rc = _start()
            if rc != 0:
                try:
                    lib.axon_stop_nrt_profile(str(output_dir).encode())
                except Exception:
                    pass
                rc = _start()
            started = rc == 0
            if not started:
                print(f"profile start failed rc={rc}; running untraced",
                      file=sys.stderr)
            try:
                yield
            finally:
                if started:
                    n = lib.axon_stop_nrt_profile(str(output_dir).encode())
                    print(f"profile: {n} file(s) -> {output_dir}",
                          file=sys.stderr)

        return _hook

    def get_axon_ntff_profile_hook():
        if mod._HOOK is None and not mod._TRIED:
            mod._TRIED = True
            p = "/opt/axon/libaxon_pjrt.so"
            if os.path.exists(p):
                try:
                    mod._HOOK = _build(p)
                except OSError:
                    mod._HOOK = None
        return mod._HOOK

    mod.set_axon_ntff_profile_hook = set_axon_ntff_profile_hook
    mod.get_axon_ntff_profile_hook = get_axon_ntff_profile_hook
    sys.modules["antenv.axon_hooks"] = mod
    antenv.axon_hooks = mod


_ensure_axon_hooks()

import concourse.bass as bass  # noqa: E402
import concourse.mybir as mybir  # noqa: E402
from concourse import tile as _tile  # noqa: E402
from concourse.tile import TileContext  # noqa: E402
from concourse.vector_clock import ScopedClock  # noqa: E402
from concourse.bass_utils import run_bass_kernel_spmd  # noqa: E402

F32 = mybir.dt.float32
F16 = mybir.dt.float16
FP8 = mybir.dt.float8e4
I32 = mybir.dt.int32
RECIP_MAGIC = float(0x7EF311C3)  # fast-inverse seed constant
WA_SCALE = 32.0                  # fp8 scale on Wa qk columns
EXP_SCALE = 1.0 / (WA_SCALE * WA_SCALE)

# problem constants (hardcoded per harness contract)
SIN_DIM, TEMBED = 64, 128
E, H, D = 512, 8, 64
B, S = 16, 1024
NCORES = 8
BPC = B // NCORES          # batches per core
S2 = BPC * S               # 2048 rows per core
J3 = 3 * H * D             # 1536
NET = E // 128             # 4 contraction tiles
NQT = S // 128             # 8 k/q tiles per batch

# ---------------------------------------------------------------------------
# Tile framework workarounds: this walrus accepts at most ONE semaphore wait
# and one update per instruction.
# ---------------------------------------------------------------------------

_NOP_CTR = [0]


def _patched_drain_and_barrier(self, tick_clock, wait_clock):
    carrier = self.nc.sync.nop(nofuse=True)
    wait_clock.add_sem_waits(
        carrier.ins, ScopedClock({None: tick_clock.global_clock})
    )
    si = carrier.ins.sync_info
    waits = list(si.on_wait) if si and si.on_wait else []
    if len(waits) > 1:
        carrier.ins.sync_info = mybir.SyncInfo(
            on_wait=waits[:1],
            on_update=list(si.on_update) if si and si.on_update else [],
        )
        for w in waits[1:]:
            extra = self.nc.sync.nop(nofuse=True)
            extra.ins.sync_info = mybir.SyncInfo(on_wait=[w], on_update=[])
    self.nc.sync.drain()
    self.nc.all_engine_barrier()
    assert self.sems is not None
    popped = self.nc._tile_sem_poison_stack.pop()
    assert popped is self._sem_poison
    self.nc.clear_and_free_semaphores(list(self.sems.allocated().values()))
    self.nc.all_engine_barrier()


_tile.TileContext._drain_and_barrier = _patched_drain_and_barrier


def _split_multi_waits(nc):
    for f in nc.m.functions:
        for blk in f.blocks:
            out = []
            changed = False
            for inst in blk.instructions:
                si = inst.sync_info
                waits = list(si.on_wait) if si and si.on_wait else []
                updates = list(si.on_update) if si and si.on_update else []
                is_dma = "DMA" in type(inst).__name__
                if len(waits) > 1:
                    changed = True
                    for w in waits[:-1]:
                        _NOP_CTR[0] += 1
                        nop = mybir.InstNoOp(
                            name=f"wsplit_{_NOP_CTR[0]}", ins=[], outs=[]
                        )
                        nop.engine = inst.engine
                        nop.sync_info = mybir.SyncInfo(on_wait=[w], on_update=[])
                        out.append(nop)
                    waits = [waits[-1]]
                    inst.sync_info = mybir.SyncInfo(
                        on_wait=waits, on_update=updates
                    )
                out.append(inst)
                if len(updates) > 1:
                    if is_dma:
                        raise AssertionError(
                            f"DMA {inst.name} has {len(updates)} updates"
                        )
                    changed = True
                    inst.sync_info = mybir.SyncInfo(
                        on_wait=waits, on_update=[updates[0]]
                    )
                    for u in updates[1:]:
                        _NOP_CTR[0] += 1
                        nop = mybir.InstNoOp(
                            name=f"usplit_{_NOP_CTR[0]}", ins=[], outs=[]
                        )
                        nop.engine = inst.engine
                        nop.sync_info = mybir.SyncInfo(on_wait=[], on_update=[u])
                        out.append(nop)
            if changed:
                blk.instructions = out
    return nc


# ---------------------------------------------------------------------------
# Single launch: attention for 2 batches per core
# ---------------------------------------------------------------------------


def build_attn(split_waits=True, salt=None):
    nc = bass.Bass()
    # fp8 operands for the QK generation: layout [128, (es=2, g=2, cols)]
    # where E-feature 256*es + 128*g + p sits at partition p.
    wa8 = nc.dram_tensor("wa8", [128, 4096], FP8, kind="ExternalInput")
    xt8 = nc.dram_tensor("xt8", [128, 8192], FP8, kind="ExternalInput")
    xt = nc.dram_tensor("xt", [E, S2], F16, kind="ExternalInput")
    wav = nc.dram_tensor("wav", [E, 512], F16, kind="ExternalInput")
    wp = nc.dram_tensor("wp", [E, E], F16, kind="ExternalInput")
    bqk = nc.dram_tensor("bqk", [128, 8], F32, kind="ExternalInput")
    bcol = nc.dram_tensor("bcol", [128, 4], F32, kind="ExternalInput")
    mask2 = nc.dram_tensor("mask2", [128, 256], F16, kind="ExternalInput")
    out = nc.dram_tensor("out", [E, S2], F16, kind="ExternalOutput")

    EXP = mybir.ActivationFunctionType.Exp
    IDENT = mybir.ActivationFunctionType.Identity
    DR = mybir.MatmulPerfMode.DoubleRow

    with TileContext(nc) as tc:
        with (
            tc.tile_pool(name="cst", bufs=1) as cst,
            tc.tile_pool(name="qk", bufs=1) as qkp,
            tc.tile_pool(name="vx", bufs=1) as vxp,
            tc.tile_pool(name="ot", bufs=1) as otp,
            tc.tile_pool(name="ex", bufs=10) as exp_pool,
            tc.tile_pool(name="nr", bufs=3) as nrp,
            tc.tile_pool(name="ob", bufs=3) as obp,
            tc.tile_pool(name="ps", bufs=3, space="PSUM") as ps,
            tc.tile_pool(name="po", bufs=1, space="PSUM") as pop,
            tc.tile_pool(name="pq", bufs=1, space="PSUM") as pqp,
        ):
            # ---- SBUF tiles ----
            wa8_t = cst.tile([128, 4096], FP8, tag="wa8", name="wa8")
            xt8_t = cst.tile([128, 8192], FP8, tag="xt8", name="xt8")
            xts = [
                cst.tile([128, S2], F16, tag=f"xt{et}", name=f"xt{et}")
                for et in range(NET)
            ]
            wavs = [
                cst.tile([128, 512], F16, tag=f"wav{et}", name=f"wav{et}")
                for et in range(NET)
            ]
            wps = [
                cst.tile([128, E], F16, tag=f"wp{et}", name=f"wp{et}")
                for et in range(NET)
            ]
            bqk_t = cst.tile([128, 8], F32)
            bcol_t = cst.tile([128, 4], F32)
            mask2_t = cst.tile([128, 256], F16)
            # v_ext tiles: [128 k-rows, 2 st x 8 h x (64 v | 64 ones)]
            vxs = {
                (b, a): vxp.tile(
                    [128, 2048], F16, tag=f"vx{b}_{a}", name=f"vx{b}_{a}"
                )
                for b in range(BPC)
                for a in range(NQT // 2)
            }
            qkts = {
                (b, m): qkp.tile(
                    [128, S], F16, tag=f"qk{b}_{m}", name=f"qkT{b}_{m}"
                )
                for b in range(BPC)
                for m in range(8)
            }
            ots = {
                (b, i): otp.tile(
                    [128, S], F16, tag=f"ot{b}_{i}", name=f"oT{b}_{i}"
                )
                for b in range(BPC)
                for i in range(4)
            }

            # ---- input DMAs (dispatch order matters: the sync engine
            # issues dma_starts serially at ~250ns each) ----
            # R0: fp8 qk-gen operands. wa8 in 4 slices; xt8 batch-0
            # columns per (es, g) block in 4 slices each.
            for a in range(4):
                nc.sync.dma_start(
                    out=wa8_t[32 * a : 32 * (a + 1), :],
                    in_=wa8[32 * a : 32 * (a + 1), :],
                )
            for eg in range(4):  # es*2 + g
                for a in range(4):
                    c0 = 2048 * eg
                    nc.sync.dma_start(
                        out=xt8_t[32 * a : 32 * (a + 1), c0 : c0 + 1024],
                        in_=xt8[32 * a : 32 * (a + 1), c0 : c0 + 1024],
                    )
            # R0.5: bias for qk evac + causal mask
            for a in range(4):
                nc.sync.dma_start(
                    out=bqk_t[32 * a : 32 * (a + 1), :],
                    in_=bqk[32 * a : 32 * (a + 1), :],
                )
                nc.sync.dma_start(
                    out=mask2_t[32 * a : 32 * (a + 1), :],
                    in_=mask2[32 * a : 32 * (a + 1), :],
                )
            # R1: f16 x (batch 0) + v-columns of wa for the v units
            for et in range(NET):
                for a in range(2):
                    nc.sync.dma_start(
                        out=xts[et][64 * a : 64 * (a + 1), 0:1024],
                        in_=xt[128 * et + 64 * a : 128 * et + 64 * (a + 1),
                               0:1024],
                    )
                    nc.sync.dma_start(
                        out=wavs[et][64 * a : 64 * (a + 1), :],
                        in_=wav[128 * et + 64 * a : 128 * et + 64 * (a + 1),
                                :],
                    )
            # R2: xt8 batch-1 columns
            for eg in range(4):
                for a in range(4):
                    c0 = 2048 * eg + 1024
                    nc.sync.dma_start(
                        out=xt8_t[32 * a : 32 * (a + 1), c0 : c0 + 1024],
                        in_=xt8[32 * a : 32 * (a + 1), c0 : c0 + 1024],
                    )
            # R3: f16 x (batch 1), wp, bcol
            for et in range(NET):
                for a in range(2):
                    nc.sync.dma_start(
                        out=xts[et][64 * a : 64 * (a + 1), 1024:2048],
                        in_=xt[128 * et + 64 * a : 128 * et + 64 * (a + 1),
                               1024:2048],
                    )
            for et in range(NET):
                for a in range(2):
                    nc.sync.dma_start(
                        out=wps[et][64 * a : 64 * (a + 1), :],
                        in_=wp[128 * et + 64 * a : 128 * et + 64 * (a + 1), :],
                    )
            for a in range(4):
                nc.sync.dma_start(
                    out=bcol_t[32 * a : 32 * (a + 1), :],
                    in_=bcol[32 * a : 32 * (a + 1), :],
                )

            # ---- ones-columns of v_ext via on-chip memset (no DMA) ----
            for b in range(BPC):
                for a in range(NQT // 2):
                    ones_ap = (
                        vxs[(b, a)][:, :]
                        .rearrange("p (s h c) -> p s h c", s=2, h=H)[
                            :, :, :, 64:128
                        ]
                    )
                    nc.vector.memset(ones_ap, 1.0)

            # ---- work units ----
            emitted = set()
            wa8_4d = wa8_t[:, :].rearrange("p (e g j) -> p e g j", e=2, g=2)
            xt8_4d = xt8_t[:, :].rearrange("p (e g s) -> p e g s", e=2, g=2)

            def qk_unit(b, m, sc):
                """qkT[m] seq-chunk sc for batch b (+ bias); fp8 DoubleRow."""
                s0 = b * S + 512 * sc
                pq0 = pqp.tile(
                    [128, 512], F32, tag="pq", name=f"pq_{b}_{m}_{sc}"
                )
                for es in range(2):
                    nc.tensor.matmul(
                        pq0,
                        wa8_4d[:, es, :, 128 * m : 128 * (m + 1)],
                        xt8_4d[:, es, :, s0 : s0 + 512],
                        start=(es == 0), stop=(es == 1),
                        perf_mode=DR,
                    )
                if sc == 0:
                    nc.scalar.activation(
                        qkts[(b, m)][:, 0:512], pq0,
                        func=IDENT, bias=bqk_t[:, m : m + 1],
                    )
                else:
                    nc.vector.tensor_scalar_add(
                        qkts[(b, m)][:, 512:1024], pq0, bqk_t[:, m : m + 1]
                    )
                emitted.add(("qk", b, m, sc))

            def v_unit(b, st):
                """v rows for seq-tile st of batch b -> v_ext v-columns."""
                pv_ = pqp.tile([128, 512], F32, tag="pq", name=f"pv_{b}_{st}")
                s0 = b * S + 128 * st
                for et in range(NET):
                    nc.tensor.matmul(
                        pv_,
                        xts[et][:, s0 : s0 + 128],
                        wavs[et][:, :],
                        start=(et == 0),
                        stop=(et == NET - 1),
                    )
                dst = (
                    vxs[(b, st // 2)][:, :]
                    .rearrange("p (s h c) -> p s h c", s=2, h=H)[
                        :, st % 2, :, 0:64
                    ]
                )
                src = pv_[:, :].rearrange("p (h c) -> p h c", h=H)
                nc.vector.tensor_copy(dst, src)
                emitted.add(("v", b, st))

            def proj_unit(b, eb, sc):
                """output e-rows [128*eb:...] for seq-chunk sc of batch b."""
                pp_ = pqp.tile(
                    [128, 512], F32, tag="pq", name=f"pp_{b}_{eb}_{sc}"
                )
                for i in range(4):
                    nc.tensor.matmul(
                        pp_,
                        wps[i][:, 128 * eb : 128 * (eb + 1)],
                        ots[(b, i)][:, 512 * sc : 512 * (sc + 1)],
                        start=(i == 0), stop=(i == 3),
                    )
                ob_ = obp.tile(
                    [128, 512], F16, tag="ob", name=f"ob_{b}_{eb}_{sc}"
                )
                if sc == 0:
                    nc.scalar.activation(
                        ob_, pp_, func=IDENT, bias=bcol_t[:, eb : eb + 1]
                    )
                else:
                    nc.vector.tensor_scalar_add(
                        ob_, pp_, bcol_t[:, eb : eb + 1]
                    )
                s0 = b * S + 512 * sc
                for a in range(4):
                    nc.sync.dma_start(
                        out=out[128 * eb + 32 * a : 128 * eb + 32 * (a + 1),
                                s0 : s0 + 512],
                        in_=ob_[32 * a : 32 * (a + 1), :],
                    )

            def attn_pair(b, i, bg, slots=None):
                """Causal attention for heads (2i, 2i+1) of batch b."""
                for mm in (i, 4 + i):
                    for sc in range(2):
                        assert ("qk", b, mm, sc) in emitted
                kt = qkts[(b, 4 + i)]
                qt = qkts[(b, i)]
                exs = {
                    j: exp_pool.tile(
                        [128, 2 * S], F16, tag="ex", name=f"ex_{b}_{i}_{j}"
                    )
                    for j in range(NQT)
                }
                # po: [128, 2048] = heads side by side; per head: o rows
                # 0-63, denominators rows 64-127; q cols 0-1023 = 2 banks
                po = pop.tile([128, 2048], F32, tag="po", name=f"po_{b}_{i}")

                bg = list(bg)
                nbg = len(bg)
                if slots is None:
                    slots = [nbg // NQT + (1 if j < nbg % NQT else 0)
                             for j in range(NQT)]
                slots = list(slots)
                assert sum(slots) == nbg
                bi = 0

                def attnv_step(hh, j):
                    h = 2 * i + hh
                    stat = vxs[(b, j // 2)][
                        :, 1024 * (j % 2) + 128 * h : 1024 * (j % 2)
                        + 128 * (h + 1)
                    ]
                    if j < 4:
                        nc.tensor.matmul(
                            po[:, 1024 * hh + 128 * j : 1024 * hh + 512],
                            stat,
                            exs[j][:, 1024 * hh + 128 * j : 1024 * hh + 512],
                            start=(j == 0), stop=(j == 3),
                        )
                    c1 = max(512, 128 * j)
                    nc.tensor.matmul(
                        po[:, 1024 * hh + c1 : 1024 * hh + 1024],
                        stat,
                        exs[j][:, 1024 * hh + c1 : 1024 * hh + 1024],
                        start=(j == 0), stop=(j == NQT - 1),
                    )

                for j in range(NQT):
                    nqc = sum(
                        1 for qc in range(2)
                        if max(512 * qc, 128 * j) < 512 * (qc + 1)
                    )
                    for qc in range(2):
                        c0 = max(512 * qc, 128 * j)
                        c1 = 512 * (qc + 1)
                        if c0 >= c1:
                            continue
                        w = c1 - c0
                        pA = ps.tile(
                            [128, 512], F32, tag="ps",
                            name=f"sA_{b}_{i}_{j}_{qc}",
                        )
                        pB = ps.tile(
                            [128, 512], F32, tag="ps",
                            name=f"sB_{b}_{i}_{j}_{qc}",
                        )
                        nc.tensor.matmul(
                            pA[:, 0:w],
                            kt[0:64, 128 * j : 128 * (j + 1)],
                            qt[0:64, c0:c1],
                            start=True, stop=True,
                        )
                        nc.tensor.matmul(
                            pB[:, 0:w],
                            kt[64:128, 128 * j : 128 * (j + 1)],
                            qt[64:128, c0:c1],
                            start=True, stop=True,
                        )
                        nc.scalar.activation(
                            exs[j][:, c0:c1], pA[:, 0:w],
                            func=EXP, scale=EXP_SCALE,
                        )
                        nc.scalar.activation(
                            exs[j][:, 1024 + c0 : 1024 + c1],
                            pB[:, 0:w],
                            func=EXP, scale=EXP_SCALE,
                        )
                        if nqc == 2 and qc == 0 and bi < nbg and slots[j] > 0:
                            bg[bi]()
                            bi += 1
                            slots[j] -= 1
                    # causal mask on the diagonal 128-col blocks (GpSimd:
                    # SBUF-only f16 work; 2D APs only)
                    nc.gpsimd.tensor_mul(
                        exs[j][:, 128 * j : 128 * (j + 1)],
                        exs[j][:, 128 * j : 128 * (j + 1)],
                        mask2_t[:, 0:128],
                    )
                    nc.gpsimd.tensor_mul(
                        exs[j][:, 1024 + 128 * j : 1024 + 128 * (j + 1)],
                        exs[j][:, 1024 + 128 * j : 1024 + 128 * (j + 1)],
                        mask2_t[:, 0:128],
                    )
                    attnv_step(0, j)
                    attnv_step(1, j)
                    for _ in range(slots[j]):
                        bg[bi]()
                        bi += 1
                assert bi == nbg

                # ---- normalize: 1/denominator via bit-trick seed + one
                # Newton step; denominators sit in po rows 64-127 ----
                sfx = f"{b}_{i}"
                s_ = nrp.tile([64, 2048], F32, tag="nr", name=f"nrs_{sfx}")
                t_ = nrp.tile([64, 2048], F32, tag="nr", name=f"nrt_{sfx}")
                u_ = nrp.tile([64, 2048], F32, tag="nr", name=f"nru_{sfx}")
                nc.vector.tensor_scalar(
                    out=s_[:, :].bitcast(I32),
                    in0=po[64:128, :].bitcast(I32),
                    scalar1=RECIP_MAGIC,
                    scalar2=-1.0,
                    op0=mybir.AluOpType.subtract,
                    op1=mybir.AluOpType.mult,
                )
                nc.vector.tensor_mul(t_, po[64:128, :], s_)
                nc.vector.scalar_tensor_tensor(
                    out=u_, in0=t_, scalar=2.0, in1=s_,
                    op0=mybir.AluOpType.subtract,
                    op1=mybir.AluOpType.mult,
                )
                for hh in range(2):
                    nc.vector.scalar_tensor_tensor(
                        out=ots[(b, i)][64 * hh : 64 * hh + 64, :],
                        in0=po[0:64, 1024 * hh : 1024 * (hh + 1)],
                        scalar=-1.0,
                        in1=u_[:, 1024 * hh : 1024 * (hh + 1)],
                        op0=mybir.AluOpType.mult,
                        op1=mybir.AluOpType.mult,
                    )

            # ---- schedule ----
            # lead-in: qk tiles for pair (0,0)
            for m in (0, 4):
                for sc in range(2):
                    qk_unit(0, m, sc)

            def mkqk(b, m, sc):
                return lambda: qk_unit(b, m, sc)

            def mkv(b, st):
                return lambda: v_unit(b, st)

            def mkp(b, eb, sc):
                return lambda: proj_unit(b, eb, sc)

            def qks(b, m):
                return [mkqk(b, m, 0), mkqk(b, m, 1)]

            bg_for_pair = {
                (0, 0): [mkv(0, 0), mkv(0, 1), mkv(0, 2), mkv(0, 3),
                         mkqk(0, 1, 0), mkqk(0, 5, 0),
                         mkv(0, 4), mkv(0, 5), mkv(0, 6), mkv(0, 7),
                         mkqk(0, 1, 1), mkqk(0, 5, 1)],
                (0, 1): qks(0, 2) + qks(0, 6),
                (0, 2): qks(0, 3) + qks(0, 7),
                (0, 3): qks(1, 0) + qks(1, 4),
                (1, 0): [mkv(1, 0), mkv(1, 1), mkv(1, 2), mkv(1, 3),
                         mkqk(1, 1, 0), mkqk(1, 5, 0),
                         mkv(1, 4), mkv(1, 5), mkv(1, 6), mkv(1, 7),
                         mkqk(1, 1, 1), mkqk(1, 5, 1)],
                (1, 1): qks(1, 2) + qks(1, 6),
                (1, 2): qks(1, 3) + qks(1, 7)
                + [mkp(0, 0, 0), mkp(0, 0, 1), mkp(0, 1, 0), mkp(0, 1, 1)],
                (1, 3): [mkp(0, 2, 0), mkp(0, 2, 1),
                         mkp(0, 3, 0), mkp(0, 3, 1)],
            }
            for b in range(BPC):
                for i in range(4):
                    attn_pair(
                        b, i, bg_for_pair[(b, i)],
                        slots=None,
                    )
            # tail: batch-1 projection (needs batch-1 normalizations,
            # which land at the end of each batch-1 pair)
            for eb in range(4):
                for sc in range(2):
                    proj_unit(1, eb, sc)
    if salt:
        nop = mybir.InstNoOp(name=f"salt_{salt}", ins=[], outs=[])
        nop.engine = mybir.EngineType.SP
        nop.sync_info = mybir.SyncInfo(on_wait=[], on_update=[])
        nc.m.functions[0].blocks[0].instructions.append(nop)
    if split_waits:
        _split_multi_waits(nc)
    return nc


# ---------------------------------------------------------------------------
# Host orchestration
# ---------------------------------------------------------------------------

_CACHE = {}


def _get(name, builder):
    if name not in _CACHE:
        _CACHE[name] = builder()
    return _CACHE[name]


def _run_with_retry(nc, in_maps, trace=False, tries=3):
    import time as _time

    last = None
    for attempt in range(tries):
        try:
            return run_bass_kernel_spmd(
                nc, in_maps, core_ids=list(range(NCORES)), trace=trace
            )
        except Exception as e:  # transient NRT_EXEC_UNIT_UNRECOVERABLE etc.
            last = e
            _time.sleep(2.0 * (attempt + 1))
    raise last


def _silu(v):
    return v / (1.0 + np.exp(-v))


def kernel(
    time_embed,
    x,
    lin1_w,
    lin1_b,
    lin2_w,
    lin2_b,
    fW_attn_w,
    fW_attn_b,
    fb_attn,
    fW_proj_w,
    fW_proj_b,
    fb_proj,
    _trace=False,
    _times=None,
):
    f64 = np.float64
    fp8np = mybir.dt.np(FP8)
    # ---- host: time-embedding MLP ----
    t1 = _silu(time_embed.astype(f64) @ lin1_w.astype(f64) + lin1_b.astype(f64))
    t = t1 @ lin2_w.astype(f64) + lin2_b.astype(f64)   # [128]
    t32 = t.astype(np.float32)

    # ---- host: hypernetwork weights ----
    Wa = t32 @ fW_attn_w.reshape(TEMBED, E * J3)
    Wa = Wa.reshape(E, J3) + fW_attn_b.reshape(E, J3)
    Wp = t32 @ fW_proj_w.reshape(TEMBED, E * E)
    Wp = Wp.reshape(E, E) + fW_proj_b.reshape(E, E)
    Wa[:, :512] *= 0.125  # fold 1/sqrt(D) into q columns

    # ---- host: fp8 qk-gen stationary: [128, (es, g, 1024 qk-cols)] ----
    Wa_qk = (Wa[:, :1024] * WA_SCALE).astype(fp8np)
    wa8_in = np.ascontiguousarray(
        Wa_qk.reshape(2, 2, 128, 1024).transpose(2, 0, 1, 3).reshape(128, 4096)
    )

    # ---- host: biases ----
    b_attn = (t @ fb_attn.astype(f64).reshape(TEMBED, J3)).astype(np.float32)
    bqk_host = b_attn[:1024].copy()
    bqk_host[:512] *= 0.125
    bqk_host *= WA_SCALE  # qkT tiles are stored x32
    bqk_in = np.ascontiguousarray(bqk_host.reshape(8, 128).T)
    b_v = b_attn[1024:]
    b_proj = (t @ fb_proj.astype(f64)).astype(np.float32)
    brow = (b_v.astype(f64) @ Wp.astype(f64) + b_proj).astype(np.float32)
    bcol_in = np.ascontiguousarray(brow.reshape(4, 128).T)
    mask1 = np.triu(np.ones((128, 128), dtype=np.float16))
    mask2_in = np.ascontiguousarray(np.concatenate([mask1, mask1], axis=1))
    Wav16 = Wa[:, 1024:1536].astype(np.float16)
    Wp16 = Wp.astype(np.float16)

    # ---- launch: attention ----
    nc_attn = _get("attn", build_attn)
    in_maps = []
    for c in range(NCORES):
        xt_c = np.ascontiguousarray(
            x[BPC * c : BPC * (c + 1)].reshape(S2, E).T
        )
        xt8_c = np.ascontiguousarray(
            xt_c.astype(fp8np)
            .reshape(2, 2, 128, S2)
            .transpose(2, 0, 1, 3)
            .reshape(128, 8192)
        )
        in_maps.append(
            {
                "wa8": wa8_in,
                "xt8": xt8_c,
                "xt": xt_c.astype(np.float16),
                "wav": Wav16,
                "wp": Wp16,
                "bqk": bqk_in,
                "bcol": bcol_in,
                "mask2": mask2_in,
            }
        )
    res = _run_with_retry(nc_attn, in_maps, trace=_trace)
    if _times is not None:
        _times.append(res.exec_time_ns)

    out = np.empty((B, S, E), dtype=np.float32)
    for c in range(NCORES):
        out[BPC * c : BPC * (c + 1)] = (
            res.results[c]["out"].astype(np.float32).T.reshape(BPC, S, E)
        )
    return out
